# revision 38
# baseline (speedup 1.0000x reference)
"""Trainium2 Bass kernel for nn_GatedFeedForward (gated feed-forward with
feature attention).

Reference computation per batch b (B=8, N=4096, D=1024):
    VR = x @ Wvr.T ; VI = x @ Wvi.T
    V  = VR * tanh(softplus(VI))
    K  = x @ Wk.T  ; Q  = x @ Wq.T
    Kn = K / (||K||_col + 1e-5) ; Qn = Q / (||Q||_col + 1e-5)   (norm over N)
    A  = smu(Kn.T @ Qn)          # smu == leaky-relu(slope 0.25) at mu=1e6
    out = V @ A
Sharding: pure data-parallel over batch — 8 batches on 8 NeuronCores.

Math restructure vs the naive 6-GEMM formulation (~45% less PE work):
  * Gram trick: K^T Q = Wk (x^T x) Wq^T.  With S = x^T x computed once,
    K and Q are never materialized:
        T' = S Wk^T ; R' = S Wq^T ; G = T'^T Wq^T   (S symmetric)
  * S is symmetric, so only the upper block-triangle is computed on the PE
    (12 of 16 [128,512] tiles); the mirrored blocks come from PE transposes.
  * Column norms from diagonals: ||K_d||^2 = diag(Wk S Wk^T)[d]
        = sum_i T'[i,d] * Wk^T[i,d]  -> elementwise product + ones-matmul.
  * smu == leaky_relu(0.25) exactly (erf saturates); leaky is positively
    homogeneous so rk folds into A rows (Prelu scale) and rq into the final
    output columns.
  * gate = tanh(softplus(vi)) evaluated as a fitted 3-term tanh mixture
        0.5 + sum_i ci * tanh(ai*vi + bi)      (max abs error 4.5e-5)
    so the scalar engine only ever uses ONE activation table in pass 2 (no
    act-table thrash from the tile scheduler) and no reciprocal is needed.

All matmuls bf16 with fp32 PSUM accumulation.
"""

import numpy as np
import ml_dtypes

import concourse.bass as bass
import concourse.tile as tile
import concourse.masks as masks
from concourse import bacc, mybir
from concourse.bass import ts

F32 = mybir.dt.float32
BF16 = mybir.dt.bfloat16
AF = mybir.ActivationFunctionType
ALU = mybir.AluOpType

B = 8
N_FULL = 4096
D_FULL = 1024
N_CORES = 8

P = 128   # SBUF partitions
NC = 512  # pass-2 sequence chunk
EF = 512  # free-dim span per matmul / psum bank

# tanh-mixture fit of tanh(softplus(x)) on [-12, 12]; limits are exact by
# construction (c1+c2+c3 = 1/2), max abs err 4.5e-5.
GC1, GA1, GB1 = 0.09744992, 0.45812075, 0.46371324
GC2, GA2, GB2 = 0.79134246, 0.60889040, -0.12094467
GC3, GA3, GB3 = 0.5 - GC1 - GC2, 0.58198337, -0.41616684


def build_program(n=N_FULL, d=D_FULL):
    """Build the single-core SPMD Bass program for one [n, d] batch."""
    assert n % NC == 0 and d % P == 0
    n_chunks = n // NC          # 8
    n_dblk = d // P             # 8 feature blocks
    n_ef = d // EF              # 2 spans of the feature dim
    n_xt = n // P               # 32 row-tiles of x

    nc = bacc.Bacc("TRN2", target_bir_lowering=False, debug=False,
                   num_devices=N_CORES)
    xn_d = nc.dram_tensor("xn", [n, d], BF16, kind="ExternalInput")
    xt_d = nc.dram_tensor("xt", [d, n], BF16, kind="ExternalInput")
    wkq_d = nc.dram_tensor("wkq", [d, 2 * d], BF16, kind="ExternalInput")
    wvr_d = nc.dram_tensor("wvr", [d, d], BF16, kind="ExternalInput")
    wvi_d = nc.dram_tensor("wvi", [d, d], BF16, kind="ExternalInput")
    out_d = nc.dram_tensor("out", [n, d], F32, kind="ExternalOutput")

    with tile.TileContext(nc) as tc:
        with tc.tile_pool(name="const", bufs=1) as const_pool, \
             tc.tile_pool(name="wv", bufs=1) as wv_pool, \
             tc.tile_pool(name="sbf", bufs=1) as sbf_pool, \
             tc.tile_pool(name="abf", bufs=1) as abf_pool, \
             tc.tile_pool(name="xt2", bufs=3) as xt_pool, \
             tc.tile_pool(name="post", bufs=1) as post_pool:

            ones_col = const_pool.tile([P, 1], BF16, name="ones_col", tag="ones_col")
            nc.vector.memset(ones_col, 1.0)
            one11 = const_pool.tile([1, 1], F32, name="one11", tag="one11")
            nc.vector.memset(one11, 1.0)
            ones_row = const_pool.tile([1, P], F32, name="ones_row", tag="ones_row")
            nc.vector.memset(ones_row, 1.0)
            ident = const_pool.tile([P, P], BF16, name="ident", tag="ident")
            masks.make_identity(nc, ident[:])
            gate_bias = [const_pool.tile([P, 1], F32, name=f"gb{i}", tag=f"gb{i}")
                         for i in range(3)]

            s_bf = [sbf_pool.tile([P, d], BF16, name=f"s{j}", tag=f"s{j}")
                    for j in range(n_dblk)]
            a_bf = [abf_pool.tile([P, d], BF16, name=f"a{j}", tag=f"a{j}")
                    for j in range(n_dblk)]

            xt_tiles = {}

            def dma_chunk(c):
                tl = []
                for dc in range(n_dblk):
                    t = xt_pool.tile([P, NC], BF16, name=f"xt{dc}", tag=f"xt{dc}")
                    nc.sync.dma_start(out=t, in_=xt_d[ts(dc, P), ts(c, NC)])
                    tl.append(t)
                xt_tiles[c] = tl

            # ---------------- Pass 1: S = x^T x (upper block-triangle) -----
            with tc.tile_pool(name="wkq", bufs=1) as wkq_pool:
                with tc.tile_pool(name="xtile", bufs=1) as xn_pool:
                    # x DMAs first — S matmuls need them immediately; weights
                    # follow in the queues (not used until P1.5/P2).  Each
                    # tile is split across two queues to halve arrival time.
                    xn = []
                    for ch in range(n_xt):
                        t = xn_pool.tile([P, d], BF16, name=f"x{ch}", tag=f"x{ch}")
                        nsplit = 4 if ch < 2 else 2  # first tiles gate startup
                        for q in range(nsplit):
                            nc.sync.dma_start(out=t[:, ts(q, d // nsplit)],
                                              in_=xn_d[ts(ch, P), ts(q, d // nsplit)])
                        xn.append(t)
                    wkq = []
                    for dc in range(n_dblk):
                        t = wkq_pool.tile([P, 2 * d], BF16, name=f"wkq{dc}", tag=f"wkq{dc}")
                        nc.sync.dma_start(out=t, in_=wkq_d[ts(dc, P), :])
                        wkq.append(t)
                    wvr, wvi = [], []
                    for wname, wdram, wl in (("wvr", wvr_d, wvr), ("wvi", wvi_d, wvi)):
                        for dc in range(n_dblk):
                            t = wv_pool.tile([P, d], BF16, name=f"{wname}{dc}", tag=f"{wname}{dc}")
                            nc.sync.dma_start(out=t, in_=wdram[ts(dc, P), :])
                            wl.append(t)
                    for t, bv in zip(gate_bias, (GB1, GB2, GB3)):
                        nc.vector.memset(t, bv)

                    def mirror(r, c, s_ps):
                        # S[r blk, c blk] = S[c blk, r blk]^T (c < r); the
                        # source lies on/above the diagonal so it was computed
                        # directly.
                        pt = s_ps.tile([P, P], BF16, name="sps_t", tag="sps")
                        nc.tensor.transpose(pt, s_bf[c][:, ts(r, P)], ident)
                        nc.vector.tensor_copy(out=s_bf[r][:, ts(c, P)], in_=pt)

                    with tc.tile_pool(name="s_ps", bufs=8, space="PSUM") as s_ps:
                        # Warm-up: dummy transposes of the identity keep the
                        # PE busy from ~1.5us while the first x tiles are in
                        # flight, so the HAM clock gate is already at 2.4GHz
                        # (and the PE start latency is paid) when the real
                        # matmuls begin.
                        for _ in range(24):
                            wt = s_ps.tile([P, P], BF16, name="warm", tag="sps")
                            nc.tensor.transpose(wt, ident, ident)
                        # Only the block upper triangle of S is computed:
                        # row-block ib covers columns [max(ib,es_lo)*128, hi).
                        # Wave B (right half) runs first — its fatter chains
                        # consume x slower, riding out the DMA cold-start —
                        # then its mirrors, then wave A (left half) as
                        # staggered chains, then wave A's mirrors.
                        # wave B: 8 interleaved chains over the right half
                        psb = {}
                        for ib in range(8):
                            c0 = max(ib, 4) * P
                            psb[ib] = s_ps.tile([P, d - c0], F32, name="sps", tag="sps")
                        for ch in range(n_xt):
                            for ib in range(8):
                                c0 = max(ib, 4) * P
                                nc.tensor.matmul(psb[ib], lhsT=xn[ch][:, ts(ib, P)],
                                                 rhs=xn[ch][:, c0:d],
                                                 start=(ch == 0), stop=(ch == n_xt - 1))
                        for ib in range(8):
                            c0 = max(ib, 4) * P
                            nc.vector.tensor_copy(out=s_bf[ib][:, c0:d], in_=psb[ib])

                        # wave A: sequential chains over the left half so the
                        # PSUM banks retire one by one.  Chain 0 is issued
                        # before the mirrors: it depends only on x, so the PE
                        # chews it while the wave-B copy burst drains.
                        def wave_a_chain(ib):
                            c0 = ib * P
                            pa = s_ps.tile([P, 4 * P - c0], F32, name="sps_a", tag="sps")
                            for ch in range(n_xt):
                                nc.tensor.matmul(pa, lhsT=xn[ch][:, ts(ib, P)],
                                                 rhs=xn[ch][:, c0:4 * P],
                                                 start=(ch == 0), stop=(ch == n_xt - 1))
                            nc.vector.tensor_copy(out=s_bf[ib][:, c0:4 * P], in_=pa)

                        wave_a_chain(0)
                        # mirrors off wave B: lower-left quadrant first (its
                        # sources are the first four wave-B copies), then
                        # rows 5-7 within the right half
                        for r in range(4, 8):
                            for c in range(4):
                                mirror(r, c, s_ps)
                        for r in range(5, 8):
                            for c in range(4, r):
                                mirror(r, c, s_ps)
                        for ib in range(1, 4):
                            wave_a_chain(ib)
                            for r in range(ib, 4):
                                mirror(r, ib - 1, s_ps)

                # ---------------- Pass 1.5: T'/R', norms, G -> A ----------
                # prefetch the first two pass-2 chunks while the DMA rings
                # are otherwise idle
                dma_chunk(0)
                dma_chunk(1)
                rk_col = post_pool.tile([P, n_dblk], F32, name="rk_col", tag="rk_col")
                rq_bc = post_pool.tile([P, d], F32, name="rq_bc", tag="rq_bc")

                with tc.tile_pool(name="tp", bufs=1) as tp_pool, \
                     tc.tile_pool(name="prod", bufs=1) as prod_pool:
                    tp_bf = [tp_pool.tile([P, d], BF16, name=f"tp{i}", tag=f"tp{i}")
                             for i in range(n_dblk)]
                    prodk = [prod_pool.tile([P, d], BF16, name=f"pk{i}", tag=f"pk{i}")
                             for i in range(n_dblk)]
                    prodq = [prod_pool.tile([P, d], BF16, name=f"pq{i}", tag=f"pq{i}")
                             for i in range(n_dblk)]

                    # T' = S Wk^T and R' = S Wq^T in one pass over wkq's 2048
                    # free columns.  T' kept in bf16 (lhsT of G); R' consumed
                    # directly from PSUM by the norm products.  Row-blocks
                    # 4..7 first: they depend only on wave-B originals, so the
                    # mirror transposes can complete in their shadow.
                    tr_order = list(range(4, 8)) + list(range(4))
                    pts_last = None
                    with tc.tile_pool(name="tr_ps", bufs=8, space="PSUM") as tr_ps:
                        for ib in tr_order:
                            pts = [tr_ps.tile([P, EF], F32, name="trps", tag="trps")
                                   for sp in range(4)]
                            for jc in range(n_dblk):
                                lh = s_bf[jc][:, ts(ib, P)]
                                for sp in range(4):
                                    nc.tensor.matmul(pts[sp], lhsT=lh,
                                                     rhs=wkq[jc][:, ts(sp, EF)],
                                                     start=(jc == 0), stop=(jc == n_dblk - 1))
                            # products feed the norm ones-matmuls.  The LAST
                            # row-block's products are deferred into the norm
                            # phase: the scheduler's coarse per-engine waits
                            # would otherwise stall the first norm matmul on
                            # them even though only the final accumulation
                            # step reads them.
                            if ib == tr_order[-1]:
                                pts_last = pts
                            else:
                                for es in range(n_ef):
                                    nc.vector.tensor_mul(out=prodk[ib][:, ts(es, EF)],
                                                         in0=pts[es],
                                                         in1=wkq[ib][:, ts(es, EF)])
                                    nc.vector.tensor_mul(out=prodq[ib][:, ts(es, EF)],
                                                         in0=pts[2 + es],
                                                         in1=wkq[ib][:, ts(2 + es, EF)])
                            # casts on the (otherwise idle) scalar engine so
                            # the vector tail after the last chain is only the
                            # norm-gating products
                            for es in range(n_ef):
                                nc.scalar.activation(out=tp_bf[ib][:, ts(es, EF)],
                                                     in_=pts[es], func=AF.Copy)

                    # column norms of K and Q via ones-matmul partition sums;
                    # reciprocals run on full-partition tiles (a [1,1024]
                    # DVE reciprocal would serialize on one lane at ~6.4us).
                    # All three PSUM pools coexist so no pool-close barrier
                    # stalls the PE between the norm chain and the G matmuls.
                    nk2 = post_pool.tile([1, d], F32, name="nk2", tag="nk2")
                    nq2 = post_pool.tile([1, d], F32, name="nq2", tag="nq2")
                    with tc.tile_pool(name="nrm_ps", bufs=2, space="PSUM") as nrm_ps, \
                         tc.tile_pool(name="misc_ps", bufs=1, space="PSUM") as misc_ps, \
                         tc.tile_pool(name="g_ps", bufs=3, space="PSUM") as g_ps:
                        last = tr_order[-1]
                        for prod, acc, base in ((prodk, nk2, 0), (prodq, nq2, 2)):
                            pns = []
                            for es in range(n_ef):
                                pn = nrm_ps.tile([1, EF], F32, name="pn", tag="nrmps")
                                for i, ic in enumerate(tr_order[:-1]):
                                    nc.tensor.matmul(pn, lhsT=ones_col,
                                                     rhs=prod[ic][:, ts(es, EF)],
                                                     start=(i == 0), stop=False)
                                pns.append(pn)
                            for es in range(n_ef):
                                nc.vector.tensor_mul(out=prod[last][:, ts(es, EF)],
                                                     in0=pts_last[base + es],
                                                     in1=wkq[last][:, ts(base + es, EF)])
                            for es in range(n_ef):
                                nc.tensor.matmul(pns[es], lhsT=ones_col,
                                                 rhs=prod[last][:, ts(es, EF)],
                                                 start=False, stop=True)
                                nc.vector.tensor_copy(out=acc[:, ts(es, EF)], in_=pns[es])

                        # rk: transpose nk2 to per-partition columns, then
                        # sqrt/recip on [128, 8] (partition-parallel).
                        nk2c = post_pool.tile([P, n_dblk], F32, name="nk2c", tag="nk2c")
                        for db in range(n_dblk):
                            pt = misc_ps.tile([P, 1], F32, name="pt", tag="miscps")
                            nc.tensor.matmul(pt, lhsT=nk2[0:1, ts(db, P)], rhs=one11,
                                             start=True, stop=True)
                            nc.vector.tensor_copy(out=nk2c[:, db:db + 1], in_=pt)
                        nc.scalar.activation(out=rk_col, in_=nk2c, func=AF.Sqrt)
                        nc.vector.tensor_scalar_add(out=rk_col, in0=rk_col, scalar1=1e-5)
                        nc.vector.reciprocal(out=rk_col, in_=rk_col)

                        # rq: broadcast nq2 across partitions first, then
                        # sqrt/recip on [128, 1024].
                        for es in range(n_ef):
                            pb = misc_ps.tile([P, EF], F32, name="pb", tag="miscps_b")
                            nc.tensor.matmul(pb, lhsT=ones_row, rhs=nq2[0:1, ts(es, EF)],
                                             start=True, stop=True)
                            nc.scalar.activation(out=rq_bc[:, ts(es, EF)], in_=pb,
                                                 func=AF.Sqrt)
                        nc.vector.tensor_scalar_add(out=rq_bc, in0=rq_bc, scalar1=1e-5)
                        nc.vector.reciprocal(out=rq_bc, in_=rq_bc)

                        # G = T'^T Wq^T ; A = Prelu(rk * G, 0.25) in bf16
                        for db in range(n_dblk):
                            for es in range(n_ef):
                                pg = g_ps.tile([P, EF], F32, name="pg", tag="gps")
                                for ic in range(n_dblk):
                                    nc.tensor.matmul(pg, lhsT=tp_bf[ic][:, ts(db, P)],
                                                     rhs=wkq[ic][:, ts(2 + es, EF)],
                                                     start=(ic == 0), stop=(ic == n_dblk - 1))
                                nc.scalar.activation(out=a_bf[db][:, ts(es, EF)], in_=pg,
                                                     func=AF.Prelu,
                                                     scale=rk_col[:, db:db + 1],
                                                     alpha=0.25)

            # ---------------- Pass 2: V, output ----------------
            with tc.tile_pool(name="vt", bufs=2) as vt_pool, \
                 tc.tile_pool(name="gtmp", bufs=10) as gtmp_pool, \
                 tc.tile_pool(name="osb", bufs=4) as osb_pool, \
                 tc.tile_pool(name="vrvi_ps", bufs=6, space="PSUM") as vrvi_ps, \
                 tc.tile_pool(name="out_ps", bufs=2, space="PSUM") as out_ps:

                vt_tiles = {}

                def vrvi_chunk(c):
                    xtt = xt_tiles.pop(c)
                    vts = []
                    for eb in range(n_dblk):
                        pvr = vrvi_ps.tile([P, NC], F32, name="pvr", tag="vrvips")
                        pvi = vrvi_ps.tile([P, NC], F32, name="pvi", tag="vrvips")
                        for dc in range(n_dblk):
                            st = dict(start=(dc == 0), stop=(dc == n_dblk - 1))
                            nc.tensor.matmul(pvr, lhsT=wvr[dc][:, ts(eb, P)], rhs=xtt[dc], **st)
                            nc.tensor.matmul(pvi, lhsT=wvi[dc][:, ts(eb, P)], rhs=xtt[dc], **st)
                        # gate = 0.5 + sum ci*tanh(ai*vi+bi); Tanh is the only
                        # scalar func in pass 2 -> single act table, no loads.
                        t1 = gtmp_pool.tile([P, NC], F32, name="t1", tag="gt")
                        t2 = gtmp_pool.tile([P, NC], F32, name="t2", tag="gt")
                        t3 = gtmp_pool.tile([P, NC], F32, name="t3", tag="gt")
                        nc.scalar.activation(out=t1, in_=pvi, func=AF.Tanh, scale=GA1,
                                             bias=gate_bias[0])
                        nc.scalar.activation(out=t2, in_=pvi, func=AF.Tanh, scale=GA2,
                                             bias=gate_bias[1])
                        nc.scalar.activation(out=t3, in_=pvi, func=AF.Tanh, scale=GA3,
                                             bias=gate_bias[2])
                        m1 = gtmp_pool.tile([P, NC], F32, name="m1", tag="gt")
                        nc.vector.scalar_tensor_tensor(out=m1, in0=t2, scalar=GC2 / GC1,
                                                       in1=t1, op0=ALU.mult, op1=ALU.add)
                        m2 = gtmp_pool.tile([P, NC], F32, name="m2", tag="gt")
                        nc.vector.scalar_tensor_tensor(out=m2, in0=t3, scalar=GC3 / GC1,
                                                       in1=m1, op0=ALU.mult, op1=ALU.add)
                        g = gtmp_pool.tile([P, NC], F32, name="g", tag="gt")
                        nc.vector.tensor_scalar(out=g, in0=m2, scalar1=GC1, scalar2=0.5,
                                                op0=ALU.mult, op1=ALU.add)
                        vt_t = vt_pool.tile([P, NC], BF16, name=f"vt{eb}", tag=f"vt{eb}")
                        nc.vector.tensor_mul(out=vt_t, in0=g, in1=pvr)
                        vts.append(vt_t)
                    vt_tiles[c] = vts

                def out_chunk(c):
                    vts = vt_tiles.pop(c)
                    for s in range(NC // P):
                        for es in range(n_ef):
                            po = out_ps.tile([P, EF], F32, name="po", tag="ops")
                            for dc in range(n_dblk):
                                nc.tensor.matmul(po, lhsT=vts[dc][:, ts(s, P)],
                                                 rhs=a_bf[dc][:, ts(es, EF)],
                                                 start=(dc == 0), stop=(dc == n_dblk - 1))
                            ot = osb_pool.tile([P, EF], F32, name="ot", tag="osb")
                            nc.vector.tensor_mul(out=ot, in0=po, in1=rq_bc[:, ts(es, EF)])
                            r0 = c * NC + s * P
                            # issue on the ACT hw-DGE: each dma_start costs
                            # ~0.6us of serial descriptor time on its issuing
                            # engine, and the sync queue is the busier one.
                            for q in range(2):
                                nc.scalar.dma_start(
                                    out=out_d[r0:r0 + P, ts(2 * es + q, EF // 2)],
                                    in_=ot[:, ts(q, EF // 2)])

                for c in range(n_chunks):
                    if c + 2 < n_chunks:
                        dma_chunk(c + 2)
                    vrvi_chunk(c)
                    if c > 0:
                        out_chunk(c - 1)
                out_chunk(n_chunks - 1)

    nc.compile()
    return nc


_PROGRAM_CACHE = {}


def _get_program(n, d):
    key = (n, d)
    if key not in _PROGRAM_CACHE:
        _PROGRAM_CACHE[key] = build_program(n, d)
    return _PROGRAM_CACHE[key]


def _numpy_reference(x, Wvr, bvr, Wvi, bvi, Wk, bk, Wq, bq):
    """Slow fp32 fallback (never expected to run: biases are zeros)."""
    out = np.empty_like(x)
    for b in range(x.shape[0]):
        xb = x[b].astype(np.float64)
        vr = xb @ Wvr.T.astype(np.float64) + bvr
        vi = xb @ Wvi.T.astype(np.float64) + bvi
        v = vr * np.tanh(np.logaddexp(0.0, vi))
        k = xb @ Wk.T.astype(np.float64) + bk
        q = xb @ Wq.T.astype(np.float64) + bq
        kn = k / (np.linalg.norm(k, axis=0, keepdims=True) + 1e-5)
        qn = q / (np.linalg.norm(q, axis=0, keepdims=True) + 1e-5)
        g = kn.T @ qn
        a = 0.625 * g + 0.375 * np.abs(g)
        out[b] = (v @ a).astype(np.float32)
    return out


def kernel(_run_kwargs=None, **inputs):
    run_kwargs = _run_kwargs or {}
    x = np.asarray(inputs["x"], dtype=np.float32)
    Wvr = np.asarray(inputs["Wvr"], dtype=np.float32)
    Wvi = np.asarray(inputs["Wvi"], dtype=np.float32)
    Wk = np.asarray(inputs["Wk"], dtype=np.float32)
    Wq = np.asarray(inputs["Wq"], dtype=np.float32)
    bvr, bvi = np.asarray(inputs["bvr"]), np.asarray(inputs["bvi"])
    bk, bq = np.asarray(inputs["bk"]), np.asarray(inputs["bq"])

    if any(np.any(b != 0) for b in (bvr, bvi, bk, bq)):
        return _numpy_reference(x, Wvr, bvr, Wvi, bvi, Wk, bk, Wq, bq)

    b, n, d = x.shape
    assert b == B and n == N_FULL and d == D_FULL, (b, n, d)

    bf16 = ml_dtypes.bfloat16
    wkq_h = np.ascontiguousarray(
        np.concatenate([Wk.T, Wq.T], axis=1)).astype(bf16)
    wvr_t = np.ascontiguousarray(Wvr.T).astype(bf16)
    wvi_t = np.ascontiguousarray(Wvi.T).astype(bf16)

    in_maps = []
    for i in range(N_CORES):
        in_maps.append({
            "xn": x[i].astype(bf16),
            "xt": np.ascontiguousarray(x[i].T).astype(bf16),
            "wkq": wkq_h, "wvr": wvr_t, "wvi": wvi_t,
        })

    nc = _get_program(n, d)
    from concourse.bass_utils import run_bass_kernel_spmd
    res = run_bass_kernel_spmd(nc, in_maps, core_ids=list(range(N_CORES)), **run_kwargs)
    out = np.stack([res.results[i]["out"] for i in range(N_CORES)], axis=0)
    if run_kwargs:
        kernel.last_results = res
    return out
